# revision 20
# baseline (speedup 1.0000x reference)
"""Bass/Trainium2 kernel for a ragged-sequence CrossAttentionBlock.

Math (per reference):
  T = 16*196 packed tokens, D=512, H=8 heads of HD=64.
  q = (xq + pos) @ Wq + bq ; k = (xk + pos) @ Wk + bk ; v = xk @ Wv + bv
  block-diagonal attention over segments of channels[i]*196 tokens
  out = softmax(q k^T / 8) v  -> concat heads -> @ Wo + bo

Sharding: one head per NeuronCore (8 heads, 8 cores). Each core computes
its head's Q/K/V over all tokens, the per-segment attention, and its
head's partial output projection (out_h @ Wo[h*64:(h+1)*64, :]). The
host sums the 8 partial projections in f32 (bo folded into core 0).

Device design notes (from trace analysis of the previous version):
 - PE matmul cost ~= out_free * ceil(out_partitions/64) cycles; AV is
   therefore run at M=64 (no ones-column) and the softmax denominators
   are computed off-tensor: vector tensor_reduce over key tiles, then a
   gpsimd partition_all_reduce, reciprocal on vector.
 - x+pos is pre-added on the host (xqp/xkp), killing the on-chip adds.
 - All host->device layouts are partition-major so every DMA descriptor
   is a contiguous 4KB per partition.
 - Issue order interleaves per-segment projections with the previous
   segments' attention; the output projection of each query block is
   deferred by one block so the normalize chain never stalls the PE.
 - Output partials are written bf16 and accumulated in f32 on host.
"""

import sys
import types

import numpy as np
import ml_dtypes

_D = 512
_HD = 64
_H = 8
_S = 196
_NCORES = 8

_prog_cache = {}


def _ensure_ntff_hook():
    """Register the NTFF profile hook that the agent image's antenv lacks."""
    if "antenv.axon_hooks" in sys.modules:
        return
    try:
        from trn_agent_boot.trn_boot import _ntff_profile_via_ctypes

        hook = _ntff_profile_via_ctypes("/opt/axon/libaxon_pjrt.so")
        mod = types.ModuleType("antenv.axon_hooks")
        mod.get_axon_ntff_profile_hook = lambda: hook
        sys.modules["antenv.axon_hooks"] = mod
    except Exception:
        pass


def _segments(channels):
    """Return (seg_len, seg_pad, seg_off, TP) for the padded token axis."""
    seg_len = [int(c) * _S for c in np.asarray(channels).tolist() if int(c) > 0]
    seg_pad = [(l + 127) // 128 * 128 for l in seg_len]
    TP = sum(seg_pad)
    TP = (TP + 511) // 512 * 512
    seg_off = []
    o = 0
    for p in seg_pad:
        seg_off.append(o)
        o += p
    return seg_len, seg_pad, seg_off, TP


def _qblocks(seg_len, seg_off):
    """Global (offset, width) query blocks of <=512 real tokens."""
    qbs = []
    for l, off in zip(seg_len, seg_off):
        for q0 in range(0, l, 512):
            qbs.append((off + q0, min(512, l - q0)))
    return qbs


def _build_program(seg_key):
    import concourse.bacc as bacc
    import concourse.tile as tile
    from concourse import mybir, bass_isa
    from concourse.masks import make_identity

    seg_len, seg_pad, seg_off, TP = seg_key[0], seg_key[1], seg_key[2], seg_key[3]
    f32 = mybir.dt.float32
    bf16 = mybir.dt.bfloat16

    NTB = TP // 512  # 512-token blocks
    NKT = TP // 128  # 128-token key tiles
    nseg = len(seg_len)
    qbs = _qblocks(seg_len, seg_off)
    NQB = len(qbs)
    max_nkt = max(p // 128 for p in seg_pad)

    nc = bacc.Bacc("TRN2", target_bir_lowering=False, debug=False, num_devices=_NCORES)

    # host layouts are partition-major: [.., 128, ..] with >=4KB contiguous
    # per partition per DMA; tensors batched to minimize dma_start count
    x3b = nc.dram_tensor("x3b", [NTB, 128, 3, 4, 512], bf16, kind="ExternalInput")
    wqkv = nc.dram_tensor("wqkv", [128, 3, 4, _HD], bf16, kind="ExternalInput")
    wo = nc.dram_tensor("wo", [_HD, _D], bf16, kind="ExternalInput")
    qkvbias = nc.dram_tensor("qkvbias", [_HD, 3], f32, kind="ExternalInput")
    obpb = nc.dram_tensor("obpb", [128, 4 + nseg], f32, kind="ExternalInput")
    outb = nc.dram_tensor("outb", [NQB, 128, 4, 512], bf16, kind="ExternalOutput")

    # blocks newly required by each segment (in-order issue)
    blk_hi = 0
    new_blocks = []
    for s in range(nseg):
        hi = (seg_off[s] + seg_pad[s] + 511) // 512
        new_blocks.append(list(range(blk_hi, hi)))
        blk_hi = hi

    with tile.TileContext(nc) as tc:
        with (
            tc.tile_pool(name="consts", bufs=1) as consts,
            tc.tile_pool(name="persist", bufs=1) as persist,
            tc.tile_pool(name="xin", bufs=1) as xin,
            tc.tile_pool(name="vtp", bufs=2) as vtp,
            tc.tile_pool(name="exp", bufs=2) as expp,
            tc.tile_pool(name="nrm", bufs=2) as nrm,
            tc.tile_pool(name="fout", bufs=2) as fout,
            tc.tile_pool(name="pp", bufs=1, space="PSUM") as pp,
            tc.tile_pool(name="po", bufs=3, space="PSUM") as po,
            tc.tile_pool(name="ptr", bufs=1, space="PSUM") as ptr,
            tc.tile_pool(name="scf", bufs=3, space="PSUM") as scf,
        ):
            # ---- constants (batched small DMAs first) ----
            wqkv_sb = consts.tile([128, 3, 4, _HD], bf16)
            wo_sb = consts.tile([_HD, _D], bf16)
            qkvb_sb = consts.tile([_HD, 3], f32)
            obpb_sb = consts.tile([128, 4 + nseg], f32)
            nc.sync.dma_start(out=wqkv_sb, in_=wqkv[:, :, :, :])
            nc.sync.dma_start(out=wo_sb, in_=wo[:, :])
            nc.sync.dma_start(out=qkvb_sb, in_=qkvbias[:, :])
            nc.sync.dma_start(out=obpb_sb, in_=obpb[:, :])
            wq_sb = wqkv_sb[:, 0]
            wk_sb = wqkv_sb[:, 1]
            wv_sb = wqkv_sb[:, 2]
            qkb_sb = qkvb_sb
            vb_sb = qkvb_sb[:, 2:3]
            ob_sb = obpb_sb[:, 0:4]
            pb_sb = obpb_sb[:, 4:4 + nseg]
            ident = consts.tile([128, 128], bf16)
            make_identity(nc, ident)

            # ---- all input DMAs up-front (stream while computing) ----
            # block 0 is split in three so the first projections can start
            # as soon as the xqp part lands
            xq_t, xk_t, xv_t = [], [], []
            for tb in range(NTB):
                t3 = xin.tile([128, 3, 4, 512], bf16, tag=f"x{tb}")
                if tb == 0:
                    for j in range(3):
                        nc.sync.dma_start(out=t3[:, j], in_=x3b[tb][:, j])
                else:
                    nc.sync.dma_start(out=t3, in_=x3b[tb])
                xq_t.append(t3[:, 0])
                xk_t.append(t3[:, 1])
                xv_t.append(t3[:, 2])

            # ---- persistent per-head tensors ----
            q_sb = persist.tile([_HD, TP], bf16)  # Q^T
            k_sb = persist.tile([_HD, TP], bf16)  # K^T
            v_sb = persist.tile([128, NKT, _HD + 1], bf16)  # V + ones col
            attn_sb = persist.tile([_HD, TP], bf16)  # normalized attention out^T
            nc.vector.memset(v_sb[:, :, _HD:_HD + 1], 1.0)

            def proj_block(tb):
                ts = slice(tb * 512, (tb + 1) * 512)
                qk_ps = pp.tile([128, 512], f32, tag="qkps")
                for c in range(4):
                    nc.tensor.matmul(
                        qk_ps[0:64, :], lhsT=wq_sb[:, c], rhs=xq_t[tb][:, c],
                        start=(c == 0), stop=(c == 3),
                        tile_position=(0, 0),
                    )
                    nc.tensor.matmul(
                        qk_ps[64:128, :], lhsT=wk_sb[:, c], rhs=xk_t[tb][:, c],
                        start=(c == 0), stop=(c == 3),
                        tile_position=(0, 64),
                    )
                vt_ps = po.tile([_HD + 1, 512], f32, tag="ops")
                for c in range(4):
                    nc.tensor.matmul(
                        vt_ps[0:_HD, :], lhsT=wv_sb[:, c], rhs=xv_t[tb][:, c],
                        start=(c == 0), stop=(c == 3),
                    )
                # evictions: q/v on vector (offset 0->0 is DVE-safe), k on
                # scalar (partition offset 64->0 only works there)
                nc.vector.tensor_scalar_add(
                    q_sb[:, ts], qk_ps[0:64, :], qkb_sb[:, 0:1]
                )
                nc.scalar.activation(
                    out=k_sb[:, ts], in_=qk_ps[64:128, :],
                    func=mybir.ActivationFunctionType.Identity,
                    bias=qkb_sb[:, 1:2],
                )
                vt_sb = vtp.tile([_HD, 512], bf16, tag="vt")
                nc.vector.tensor_scalar_add(
                    vt_sb, vt_ps[0:_HD, :], vb_sb
                )
                tr_ps = ptr.tile([128, 4, _HD], bf16, tag="tr")
                for i in range(4):
                    nc.tensor.transpose(
                        tr_ps[:, i, :], vt_sb[:, i * 128:(i + 1) * 128],
                        ident[0:64, 0:64],
                    )
                nc.vector.tensor_copy(
                    out=v_sb[:, 4 * tb:4 * (tb + 1), 0:_HD], in_=tr_ps
                )

            # deferred output projection state: one query block behind
            pending = []

            def flush_outproj():
                while pending:
                    qbi, q0, qw = pending.pop(0)
                    f_sb = fout.tile([128, 4, 512], bf16, tag="fsb")
                    for ec in range(4):
                        f_ps = scf.tile([128, 512], f32, tag="sc")
                        nc.tensor.matmul(
                            f_ps[:, 0:qw],
                            lhsT=wo_sb[:, ec * 128:(ec + 1) * 128],
                            rhs=attn_sb[:, q0:q0 + qw],
                            start=True, stop=True,
                        )
                        if ec % 2 == 0:
                            nc.vector.tensor_scalar_add(
                                f_sb[:, ec, 0:qw], f_ps[:, 0:qw],
                                ob_sb[:, ec:ec + 1],
                            )
                        else:
                            nc.scalar.activation(
                                out=f_sb[:, ec, 0:qw], in_=f_ps[:, 0:qw],
                                func=mybir.ActivationFunctionType.Identity,
                                bias=ob_sb[:, ec:ec + 1],
                            )
                    nc.sync.dma_start(out=outb[qbi], in_=f_sb)

            # block projection queue: (deadline segment, block); issued
            # opportunistically inside the attention stream as PE-gap filler
            blockq = []
            for s in range(nseg):
                for tb in new_blocks[s]:
                    blockq.append((s, tb))

            qblist = []
            for s in range(nseg):
                off, real = seg_off[s], seg_len[s]
                for b in range(0, real, 512):
                    qblist.append((s, off + b, min(512, real - b)))
            ex_of = {}

            def do_scores(i):
                s, q0, qw = qblist[i]
                off, L, real = seg_off[s], seg_pad[s], seg_len[s]
                nkt = L // 128
                ex = expp.tile([128, max_nkt, 512], bf16, tag="ex")
                ex_of[i] = ex
                for kt in range(nkt):
                    klo = off + kt * 128
                    sc_ps = scf.tile([128, 512], f32, tag="sc")
                    nc.tensor.matmul(
                        sc_ps[:, 0:qw],
                        lhsT=k_sb[:, klo:klo + 128],
                        rhs=q_sb[:, q0:q0 + qw],
                        start=True, stop=True,
                    )
                    is_pad = kt == nkt - 1 and real < L
                    nc.scalar.activation(
                        out=ex[:, kt, 0:qw], in_=sc_ps[:, 0:qw],
                        func=mybir.ActivationFunctionType.Exp,
                        scale=0.125,
                        bias=pb_sb[:, s:s + 1] if is_pad else 0.0,
                    )

            def do_av(i):
                s, q0, qw = qblist[i]
                off, L = seg_off[s], seg_pad[s]
                nkt = L // 128
                kt0 = off // 128
                ex = ex_of.pop(i)
                # A@V; ones column in V row 64 gives the denominators
                o_ps = po.tile([_HD + 1, 512], f32, tag="ops")
                for kt in range(nkt):
                    nc.tensor.matmul(
                        o_ps[:, 0:qw],
                        lhsT=v_sb[:, kt0 + kt, :],
                        rhs=ex[:, kt, 0:qw],
                        start=(kt == 0), stop=(kt == nkt - 1),
                    )
                sums = nrm.tile([1, 512], f32, tag="sums")
                nc.scalar.copy(out=sums[:, 0:qw], in_=o_ps[_HD:_HD + 1, 0:qw])
                rec1 = nrm.tile([1, 512], f32, tag="rec1")
                rs1 = nrm.tile([1, 512], f32, tag="rs1")
                nc.vector.reciprocal_approx_accurate(
                    out=rec1[:, 0:qw], in_=sums[:, 0:qw],
                    scratch=rs1[:, 0:qw],
                )
                bc = nrm.tile([_HD, 512], f32, tag="bc")
                nc.gpsimd.partition_broadcast(bc[:, 0:qw], rec1[:, 0:qw])
                nc.vector.tensor_mul(
                    attn_sb[:, q0:q0 + qw], o_ps[0:_HD, 0:qw], bc[:, 0:qw]
                )
                pending.append((i, q0, qw))

            # two-stage software pipeline over query blocks: scores(i) run
            # while exp/AV of i-1 and the output projection of i-2 drain,
            # with input-projection blocks woven in as extra filler
            for i in range(len(qblist)):
                s = qblist[i][0]
                while blockq and blockq[0][0] <= s:
                    proj_block(blockq.pop(0)[1])
                do_scores(i)
                flush_outproj()
                if blockq:
                    proj_block(blockq.pop(0)[1])
                if i > 0:
                    do_av(i - 1)
            do_av(len(qblist) - 1)
            flush_outproj()

    nc.compile()
    return nc


def _prep_token_major(x, seg_len, seg_pad, seg_off, TP):
    """[T, D] f32 -> [NTB, 128, 4, 512] bf16, partition-major contiguous."""
    xp = np.zeros((TP, _D), dtype=np.float32)
    o = 0
    for l, p, off in zip(seg_len, seg_pad, seg_off):
        xp[off:off + l] = x[o:o + l]
        o += l
    xt = np.ascontiguousarray(xp.T)  # [D, TP]
    h = xt.reshape(4, 128, TP // 512, 512).transpose(2, 1, 0, 3)
    return np.ascontiguousarray(h).astype(ml_dtypes.bfloat16)


def kernel(x_query, x_keyval, pos, channels, Wq, bq, Wk, bk, Wv, bv, Wo, bo,
           _trace=False, _trace_cores=None):
    _ensure_ntff_hook()
    import concourse.bass_utils as bu

    bu.upload_artifacts = lambda tmpdir: tmpdir  # no S3 egress from here

    x_query = np.asarray(x_query, dtype=np.float32)
    x_keyval = np.asarray(x_keyval, dtype=np.float32)
    pos = np.asarray(pos, dtype=np.float32)
    channels = np.asarray(channels)
    Wq, bq = np.asarray(Wq, np.float32), np.asarray(bq, np.float32)
    Wk, bk = np.asarray(Wk, np.float32), np.asarray(bk, np.float32)
    Wv, bv = np.asarray(Wv, np.float32), np.asarray(bv, np.float32)
    Wo, bo = np.asarray(Wo, np.float32), np.asarray(bo, np.float32)

    C, S, D = x_query.shape
    seg_len, seg_pad, seg_off, TP = _segments(channels)
    assert sum(seg_len) == C * S, "channels inconsistent with batch dim"

    seg_key = (tuple(seg_len), tuple(seg_pad), tuple(seg_off), TP)
    if seg_key not in _prog_cache:
        _prog_cache[seg_key] = _build_program(seg_key)
    nc = _prog_cache[seg_key]

    bf = ml_dtypes.bfloat16
    xq2 = (x_query + pos).reshape(-1, D)
    xk2 = (x_keyval + pos).reshape(-1, D)
    xqpb = _prep_token_major(xq2, seg_len, seg_pad, seg_off, TP)
    xkpb = _prep_token_major(xk2, seg_len, seg_pad, seg_off, TP)
    xkvb = _prep_token_major(x_keyval.reshape(-1, D), seg_len, seg_pad, seg_off, TP)
    # batched input: [NTB, 128, 3(xqp,xkp,xkv), 4, 512]
    x3b = np.ascontiguousarray(
        np.stack([xqpb, xkpb, xkvb], axis=2)
    )

    # per-segment pad bias: -87 on padded key rows of the segment's last k-tile
    nseg = len(seg_len)
    padbias = np.zeros((128, nseg), dtype=np.float32)
    for s in range(nseg):
        plo = seg_len[s] - (seg_pad[s] // 128 - 1) * 128
        if plo < 128:
            padbias[plo:, s] = -87.0

    in_maps = []
    for h in range(_NCORES):
        sl = slice(h * _HD, (h + 1) * _HD)
        ob = bo if h == 0 else np.zeros_like(bo)
        wqkv = np.stack(
            [W[:, sl].reshape(4, 128, _HD).transpose(1, 0, 2)
             for W in (Wq, Wk, Wv)], axis=1)
        obpb = np.concatenate(
            [ob.reshape(4, 128).T.astype(np.float32), padbias], axis=1)
        in_maps.append({
            "x3b": x3b,
            "wqkv": np.ascontiguousarray(wqkv).astype(bf),
            "wo": np.ascontiguousarray(Wo[sl, :]).astype(bf),
            "qkvbias": np.ascontiguousarray(
                np.stack([bq[sl], bk[sl], bv[sl]], axis=1)),
            "obpb": np.ascontiguousarray(obpb),
        })

    from concourse.bass_utils import run_bass_kernel_spmd

    kwargs = {}
    if _trace:
        kwargs["trace"] = True
        if _trace_cores is not None:
            kwargs["trace_cores"] = _trace_cores
    res = run_bass_kernel_spmd(nc, in_maps, list(range(_NCORES)), **kwargs)

    qbs = _qblocks(seg_len, seg_off)
    acc = np.zeros((len(qbs), 128, 4, 512), dtype=np.float32)
    for h in range(_NCORES):
        acc += res.results[h]["outb"].astype(np.float32)

    # reassemble [512, TP] then unpad + transpose back
    outT = np.zeros((_D, TP), dtype=np.float32)
    for i, (q0, qw) in enumerate(qbs):
        blk = acc[i].transpose(1, 0, 2).reshape(_D, 512)
        outT[:, q0:q0 + qw] = blk[:, 0:qw]

    out = np.empty((C * S, D), dtype=np.float32)
    o = 0
    for l, off in zip(seg_len, seg_off):
        out[o:o + l] = outT[:, off:off + l].T
        o += l
    out = out.reshape(C, S, D)

    if _trace:
        kernel._last_exec_time_ns = res.exec_time_ns
        kernel._last_trace = (
            res.instructions_and_trace[1] if res.instructions_and_trace else None
        )
    return out


# revision 23
# speedup vs baseline: 1.0166x; 1.0166x over previous
"""Bass/Trainium2 kernel for a ragged-sequence CrossAttentionBlock.

Math (per reference):
  T = 16*196 packed tokens, D=512, H=8 heads of HD=64.
  q = (xq + pos) @ Wq + bq ; k = (xk + pos) @ Wk + bk ; v = xk @ Wv + bv
  block-diagonal attention over segments of channels[i]*196 tokens
  out = softmax(q k^T / 8) v  -> concat heads -> @ Wo + bo

Sharding: one head per NeuronCore (8 heads, 8 cores). Each core computes
its head's Q/K/V over all tokens, the per-segment attention, and its
head's partial output projection (out_h @ Wo[h*64:(h+1)*64, :]). The
host sums the 8 partial projections in f32 (bo folded into core 0).

Device design notes (from trace analysis of the previous version):
 - PE matmul cost ~= out_free * ceil(out_partitions/64) cycles; AV is
   therefore run at M=64 (no ones-column) and the softmax denominators
   are computed off-tensor: vector tensor_reduce over key tiles, then a
   gpsimd partition_all_reduce, reciprocal on vector.
 - x+pos is pre-added on the host (xqp/xkp), killing the on-chip adds.
 - All host->device layouts are partition-major so every DMA descriptor
   is a contiguous 4KB per partition.
 - Issue order interleaves per-segment projections with the previous
   segments' attention; the output projection of each query block is
   deferred by one block so the normalize chain never stalls the PE.
 - Output partials are written bf16 and accumulated in f32 on host.
"""

import sys
import types

import numpy as np
import ml_dtypes

_D = 512
_HD = 64
_H = 8
_S = 196
_NCORES = 8

_prog_cache = {}


def _ensure_ntff_hook():
    """Register the NTFF profile hook that the agent image's antenv lacks."""
    if "antenv.axon_hooks" in sys.modules:
        return
    try:
        from trn_agent_boot.trn_boot import _ntff_profile_via_ctypes

        hook = _ntff_profile_via_ctypes("/opt/axon/libaxon_pjrt.so")
        mod = types.ModuleType("antenv.axon_hooks")
        mod.get_axon_ntff_profile_hook = lambda: hook
        sys.modules["antenv.axon_hooks"] = mod
    except Exception:
        pass


def _segments(channels):
    """Return (seg_len, seg_pad, seg_off, TP) for the padded token axis."""
    seg_len = [int(c) * _S for c in np.asarray(channels).tolist() if int(c) > 0]
    seg_pad = [(l + 127) // 128 * 128 for l in seg_len]
    TP = sum(seg_pad)
    TP = (TP + 511) // 512 * 512
    seg_off = []
    o = 0
    for p in seg_pad:
        seg_off.append(o)
        o += p
    return seg_len, seg_pad, seg_off, TP


def _qblocks(seg_len, seg_off):
    """Global (offset, width) query blocks of <=512 real tokens."""
    qbs = []
    for l, off in zip(seg_len, seg_off):
        for q0 in range(0, l, 512):
            qbs.append((off + q0, min(512, l - q0)))
    return qbs


def _build_program(seg_key):
    import concourse.bacc as bacc
    import concourse.tile as tile
    from concourse import mybir, bass_isa
    from concourse.masks import make_identity

    seg_len, seg_pad, seg_off, TP = seg_key[0], seg_key[1], seg_key[2], seg_key[3]
    f32 = mybir.dt.float32
    bf16 = mybir.dt.bfloat16

    NTB = TP // 512  # 512-token blocks
    NKT = TP // 128  # 128-token key tiles
    nseg = len(seg_len)
    qbs = _qblocks(seg_len, seg_off)
    NQB = len(qbs)
    max_nkt = max(p // 128 for p in seg_pad)

    nc = bacc.Bacc("TRN2", target_bir_lowering=False, debug=False, num_devices=_NCORES)

    # host layouts are partition-major: [.., 128, ..] with >=4KB contiguous
    # per partition per DMA; tensors batched to minimize dma_start count
    x3b = nc.dram_tensor("x3b", [NTB, 128, 3, 4, 512], bf16, kind="ExternalInput")
    wqkv = nc.dram_tensor("wqkv", [128, 3, 4, _HD], bf16, kind="ExternalInput")
    wo = nc.dram_tensor("wo", [_HD, _D], bf16, kind="ExternalInput")
    qkvbias = nc.dram_tensor("qkvbias", [_HD, 3], f32, kind="ExternalInput")
    obpb = nc.dram_tensor("obpb", [128, 4 + nseg], f32, kind="ExternalInput")
    outb = nc.dram_tensor("outb", [NQB, 128, 4, 512], bf16, kind="ExternalOutput")

    # blocks newly required by each segment (in-order issue)
    blk_hi = 0
    new_blocks = []
    for s in range(nseg):
        hi = (seg_off[s] + seg_pad[s] + 511) // 512
        new_blocks.append(list(range(blk_hi, hi)))
        blk_hi = hi

    with tile.TileContext(nc) as tc:
        with (
            tc.tile_pool(name="consts", bufs=1) as consts,
            tc.tile_pool(name="persist", bufs=1) as persist,
            tc.tile_pool(name="xin", bufs=1) as xin,
            tc.tile_pool(name="vtp", bufs=2) as vtp,
            tc.tile_pool(name="exp", bufs=2) as expp,
            tc.tile_pool(name="nrm", bufs=2) as nrm,
            tc.tile_pool(name="fout", bufs=2) as fout,
            tc.tile_pool(name="pp", bufs=1, space="PSUM") as pp,
            tc.tile_pool(name="po", bufs=3, space="PSUM") as po,
            tc.tile_pool(name="ptr", bufs=1, space="PSUM") as ptr,
            tc.tile_pool(name="scf", bufs=3, space="PSUM") as scf,
        ):
            # ---- constants (batched small DMAs first) ----
            wqkv_sb = consts.tile([128, 3, 4, _HD], bf16)
            wo_sb = consts.tile([_HD, _D], bf16)
            qkvb_sb = consts.tile([_HD, 3], f32)
            obpb_sb = consts.tile([128, 4 + nseg], f32)
            nc.sync.dma_start(out=wqkv_sb, in_=wqkv[:, :, :, :])
            nc.sync.dma_start(out=wo_sb, in_=wo[:, :])
            nc.sync.dma_start(out=qkvb_sb, in_=qkvbias[:, :])
            nc.sync.dma_start(out=obpb_sb, in_=obpb[:, :])
            wq_sb = wqkv_sb[:, 0]
            wk_sb = wqkv_sb[:, 1]
            wv_sb = wqkv_sb[:, 2]
            qkb_sb = qkvb_sb
            vb_sb = qkvb_sb[:, 2:3]
            ob_sb = obpb_sb[:, 0:4]
            pb_sb = obpb_sb[:, 4:4 + nseg]
            ident = consts.tile([128, 128], bf16)
            make_identity(nc, ident)

            # ---- all input DMAs up-front (stream while computing) ----
            # block 0 is split in three so the first projections can start
            # as soon as the xqp part lands
            xq_t, xk_t, xv_t = [], [], []
            for tb in range(NTB):
                t3 = xin.tile([128, 3, 4, 512], bf16, tag=f"x{tb}")
                if tb == 0:
                    for j in range(3):
                        nc.sync.dma_start(out=t3[:, j], in_=x3b[tb][:, j])
                else:
                    nc.sync.dma_start(out=t3, in_=x3b[tb])
                xq_t.append(t3[:, 0])
                xk_t.append(t3[:, 1])
                xv_t.append(t3[:, 2])

            # ---- persistent per-head tensors ----
            q_sb = persist.tile([_HD, TP], bf16)  # Q^T
            k_sb = persist.tile([_HD, TP], bf16)  # K^T
            v_sb = persist.tile([128, NKT, _HD + 1], bf16)  # V + ones col
            attn_sb = persist.tile([_HD, TP], bf16)  # normalized attention out^T
            nc.vector.memset(v_sb[:, :, _HD:_HD + 1], 1.0)

            def proj_block(tb):
                ts = slice(tb * 512, (tb + 1) * 512)
                qk_ps = pp.tile([128, 512], f32, tag="qkps")
                for c in range(4):
                    nc.tensor.matmul(
                        qk_ps[0:64, :], lhsT=wq_sb[:, c], rhs=xq_t[tb][:, c],
                        start=(c == 0), stop=(c == 3),
                        tile_position=(0, 0),
                    )
                    nc.tensor.matmul(
                        qk_ps[64:128, :], lhsT=wk_sb[:, c], rhs=xk_t[tb][:, c],
                        start=(c == 0), stop=(c == 3),
                        tile_position=(0, 64),
                    )
                vt_ps = po.tile([_HD + 1, 512], f32, tag="ops")
                for c in range(4):
                    nc.tensor.matmul(
                        vt_ps[0:_HD, :], lhsT=wv_sb[:, c], rhs=xv_t[tb][:, c],
                        start=(c == 0), stop=(c == 3),
                    )
                # evictions: q/v on vector (offset 0->0 is DVE-safe), k on
                # scalar (partition offset 64->0 only works there)
                nc.vector.tensor_scalar_add(
                    q_sb[:, ts], qk_ps[0:64, :], qkb_sb[:, 0:1]
                )
                nc.scalar.activation(
                    out=k_sb[:, ts], in_=qk_ps[64:128, :],
                    func=mybir.ActivationFunctionType.Identity,
                    bias=qkb_sb[:, 1:2],
                )
                vt_sb = vtp.tile([_HD, 512], bf16, tag="vt")
                nc.vector.tensor_scalar_add(
                    vt_sb, vt_ps[0:_HD, :], vb_sb
                )
                tr_ps = ptr.tile([128, 4, _HD], bf16, tag="tr")
                for i in range(4):
                    nc.tensor.transpose(
                        tr_ps[:, i, :], vt_sb[:, i * 128:(i + 1) * 128],
                        ident[0:64, 0:64],
                    )
                nc.vector.tensor_copy(
                    out=v_sb[:, 4 * tb:4 * (tb + 1), 0:_HD], in_=tr_ps
                )

            # deferred output projection state: one query block behind
            pending = []

            def flush_outproj():
                while pending:
                    qbi, q0, qw = pending.pop(0)
                    f_sb = fout.tile([128, 4, 512], bf16, tag="fsb")
                    for ec in range(4):
                        f_ps = scf.tile([128, 512], f32, tag="sc")
                        nc.tensor.matmul(
                            f_ps[:, 0:qw],
                            lhsT=wo_sb[:, ec * 128:(ec + 1) * 128],
                            rhs=attn_sb[:, q0:q0 + qw],
                            start=True, stop=True,
                        )
                        if ec % 2 == 0:
                            nc.vector.tensor_scalar_add(
                                f_sb[:, ec, 0:qw], f_ps[:, 0:qw],
                                ob_sb[:, ec:ec + 1],
                            )
                        else:
                            nc.scalar.activation(
                                out=f_sb[:, ec, 0:qw], in_=f_ps[:, 0:qw],
                                func=mybir.ActivationFunctionType.Identity,
                                bias=ob_sb[:, ec:ec + 1],
                            )
                    nc.sync.dma_start(out=outb[qbi], in_=f_sb)

            # block projection queue: (deadline segment, block); issued
            # opportunistically inside the attention stream as PE-gap filler
            blockq = []
            for s in range(nseg):
                for tb in new_blocks[s]:
                    blockq.append((s, tb))

            qblist = []
            for s in range(nseg):
                off, real = seg_off[s], seg_len[s]
                for b in range(0, real, 512):
                    qblist.append((s, off + b, min(512, real - b)))
            ex_of = {}

            def do_scores(i):
                s, q0, qw = qblist[i]
                off, L, real = seg_off[s], seg_pad[s], seg_len[s]
                nkt = L // 128
                ex = expp.tile([128, max_nkt, 512], bf16, tag="ex")
                ex_of[i] = ex
                for kt in range(nkt):
                    klo = off + kt * 128
                    sc_ps = scf.tile([128, 512], f32, tag="sc")
                    nc.tensor.matmul(
                        sc_ps[:, 0:qw],
                        lhsT=k_sb[:, klo:klo + 128],
                        rhs=q_sb[:, q0:q0 + qw],
                        start=True, stop=True,
                    )
                    is_pad = kt == nkt - 1 and real < L
                    nc.scalar.activation(
                        out=ex[:, kt, 0:qw], in_=sc_ps[:, 0:qw],
                        func=mybir.ActivationFunctionType.Exp,
                        scale=0.125,
                        bias=pb_sb[:, s:s + 1] if is_pad else 0.0,
                    )

            ops_of = {}

            def do_av(i):
                s, q0, qw = qblist[i]
                off, L = seg_off[s], seg_pad[s]
                nkt = L // 128
                kt0 = off // 128
                ex = ex_of.pop(i)
                # A@V; ones column in V row 64 gives the denominators
                o_ps = po.tile([_HD + 1, 512], f32, tag="ops")
                ops_of[i] = o_ps
                for kt in range(nkt):
                    nc.tensor.matmul(
                        o_ps[:, 0:qw],
                        lhsT=v_sb[:, kt0 + kt, :],
                        rhs=ex[:, kt, 0:qw],
                        start=(kt == 0), stop=(kt == nkt - 1),
                    )

            def do_norm(i):
                s, q0, qw = qblist[i]
                o_ps = ops_of.pop(i)
                sums = nrm.tile([1, 512], f32, tag="sums")
                nc.scalar.copy(out=sums[:, 0:qw], in_=o_ps[_HD:_HD + 1, 0:qw])
                rec1 = nrm.tile([1, 512], f32, tag="rec1")
                rs1 = nrm.tile([1, 512], f32, tag="rs1")
                nc.vector.reciprocal_approx_accurate(
                    out=rec1[:, 0:qw], in_=sums[:, 0:qw], scratch=rs1[:, 0:qw],
                )
                bc = nrm.tile([_HD, 512], f32, tag="bc")
                nc.gpsimd.partition_broadcast(bc[:, 0:qw], rec1[:, 0:qw])
                nc.vector.tensor_mul(
                    attn_sb[:, q0:q0 + qw], o_ps[0:_HD, 0:qw], bc[:, 0:qw]
                )
                pending.append((i, q0, qw))

            # three-stage software pipeline over query blocks: scores(i),
            # A@V(i-1), normalize(i-2), outproj(i-3). The scalar engine only
            # sees exp bursts plus a denominator copy whose AV finished a
            # full iteration earlier, so it never back-pressures the PE.
            nqb = len(qblist)
            for i in range(nqb):
                s = qblist[i][0]
                while blockq and blockq[0][0] <= s:
                    proj_block(blockq.pop(0)[1])
                do_scores(i)
                flush_outproj()
                if blockq:
                    proj_block(blockq.pop(0)[1])
                if i > 0:
                    do_av(i - 1)
                if i > 1:
                    do_norm(i - 2)
            do_av(nqb - 1)
            do_norm(nqb - 2)
            flush_outproj()
            do_norm(nqb - 1)
            flush_outproj()

    nc.compile()
    return nc


def _prep_token_major(x, seg_len, seg_pad, seg_off, TP):
    """[T, D] f32 -> [NTB, 128, 4, 512] bf16, partition-major contiguous."""
    xp = np.zeros((TP, _D), dtype=np.float32)
    o = 0
    for l, p, off in zip(seg_len, seg_pad, seg_off):
        xp[off:off + l] = x[o:o + l]
        o += l
    xt = np.ascontiguousarray(xp.T)  # [D, TP]
    h = xt.reshape(4, 128, TP // 512, 512).transpose(2, 1, 0, 3)
    return np.ascontiguousarray(h).astype(ml_dtypes.bfloat16)


def kernel(x_query, x_keyval, pos, channels, Wq, bq, Wk, bk, Wv, bv, Wo, bo,
           _trace=False, _trace_cores=None):
    _ensure_ntff_hook()
    import concourse.bass_utils as bu

    bu.upload_artifacts = lambda tmpdir: tmpdir  # no S3 egress from here

    x_query = np.asarray(x_query, dtype=np.float32)
    x_keyval = np.asarray(x_keyval, dtype=np.float32)
    pos = np.asarray(pos, dtype=np.float32)
    channels = np.asarray(channels)
    Wq, bq = np.asarray(Wq, np.float32), np.asarray(bq, np.float32)
    Wk, bk = np.asarray(Wk, np.float32), np.asarray(bk, np.float32)
    Wv, bv = np.asarray(Wv, np.float32), np.asarray(bv, np.float32)
    Wo, bo = np.asarray(Wo, np.float32), np.asarray(bo, np.float32)

    C, S, D = x_query.shape
    seg_len, seg_pad, seg_off, TP = _segments(channels)
    assert sum(seg_len) == C * S, "channels inconsistent with batch dim"

    seg_key = (tuple(seg_len), tuple(seg_pad), tuple(seg_off), TP)
    if seg_key not in _prog_cache:
        _prog_cache[seg_key] = _build_program(seg_key)
    nc = _prog_cache[seg_key]

    bf = ml_dtypes.bfloat16
    xq2 = (x_query + pos).reshape(-1, D)
    xk2 = (x_keyval + pos).reshape(-1, D)
    xqpb = _prep_token_major(xq2, seg_len, seg_pad, seg_off, TP)
    xkpb = _prep_token_major(xk2, seg_len, seg_pad, seg_off, TP)
    xkvb = _prep_token_major(x_keyval.reshape(-1, D), seg_len, seg_pad, seg_off, TP)
    # batched input: [NTB, 128, 3(xqp,xkp,xkv), 4, 512]
    x3b = np.ascontiguousarray(
        np.stack([xqpb, xkpb, xkvb], axis=2)
    )

    # per-segment pad bias: -87 on padded key rows of the segment's last k-tile
    nseg = len(seg_len)
    padbias = np.zeros((128, nseg), dtype=np.float32)
    for s in range(nseg):
        plo = seg_len[s] - (seg_pad[s] // 128 - 1) * 128
        if plo < 128:
            padbias[plo:, s] = -87.0

    in_maps = []
    for h in range(_NCORES):
        sl = slice(h * _HD, (h + 1) * _HD)
        ob = bo if h == 0 else np.zeros_like(bo)
        wqkv = np.stack(
            [W[:, sl].reshape(4, 128, _HD).transpose(1, 0, 2)
             for W in (Wq, Wk, Wv)], axis=1)
        obpb = np.concatenate(
            [ob.reshape(4, 128).T.astype(np.float32), padbias], axis=1)
        in_maps.append({
            "x3b": x3b,
            "wqkv": np.ascontiguousarray(wqkv).astype(bf),
            "wo": np.ascontiguousarray(Wo[sl, :]).astype(bf),
            "qkvbias": np.ascontiguousarray(
                np.stack([bq[sl], bk[sl], bv[sl]], axis=1)),
            "obpb": np.ascontiguousarray(obpb),
        })

    from concourse.bass_utils import run_bass_kernel_spmd

    kwargs = {}
    if _trace:
        kwargs["trace"] = True
        if _trace_cores is not None:
            kwargs["trace_cores"] = _trace_cores
    res = run_bass_kernel_spmd(nc, in_maps, list(range(_NCORES)), **kwargs)

    qbs = _qblocks(seg_len, seg_off)
    acc = np.zeros((len(qbs), 128, 4, 512), dtype=np.float32)
    for h in range(_NCORES):
        acc += res.results[h]["outb"].astype(np.float32)

    # reassemble [512, TP] then unpad + transpose back
    outT = np.zeros((_D, TP), dtype=np.float32)
    for i, (q0, qw) in enumerate(qbs):
        blk = acc[i].transpose(1, 0, 2).reshape(_D, 512)
        outT[:, q0:q0 + qw] = blk[:, 0:qw]

    out = np.empty((C * S, D), dtype=np.float32)
    o = 0
    for l, off in zip(seg_len, seg_off):
        out[o:o + l] = outT[:, off:off + l].T
        o += l
    out = out.reshape(C, S, D)

    if _trace:
        kernel._last_exec_time_ns = res.exec_time_ns
        kernel._last_trace = (
            res.instructions_and_trace[1] if res.instructions_and_trace else None
        )
    return out
